# revision 23
# baseline (speedup 1.0000x reference)
"""Trainium2 Bass kernel for nn_NestedGraphTitanV6 (gated delta-rule memory net).

Sharding: data-parallel over B across 8 cores (B_loc=8 -> 2048 tokens,
32 recurrence sequences of length 64 per core; the torch-style .view reshape
makes consecutive 64-token blocks the scan sequences, so no data movement).

Layout: feature-major activations [d=128 partitions, tokens free].

Scan: with M0=0 and eta in [0.01, 0.11], the S=64 delta-rule scan is computed
in closed form via the WY representation; (I+L)^{-1} ~= (I-L) with L
strictly-lower [64x64] (truncation validated: final output rel err ~5e-7 vs
the oracle, measured on hardware).

Dtypes: fp32 GEMMs (this walrus build rejects bitcast-fp32r operands);
bf16 for the per-sequence 64x64 chain. LayerNorm stats use an all-ones
stationary matmul so the sum arrives broadcast to all partitions in one op.
"""
import numpy as np

import concourse.bass as bass
import concourse.mybir as mybir
import concourse.tile as tile
from concourse.masks import make_identity

F32 = mybir.dt.float32
F32R = mybir.dt.float32r
BF16 = mybir.dt.bfloat16
AF = mybir.ActivationFunctionType
ALU = mybir.AluOpType
AX = mybir.AxisListType

B, S, N, FIN, D = 64, 64, 4, 23, 128
NCORES = 8
BL = B // NCORES      # 8
T = BL * S * N        # 2048 tokens / core
R = BL * N            # 32 sequences / core
CH = 512              # token chunk
NCH = T // CH
RSQ = float(1.0 / np.sqrt(np.float32(128.0)))

_HEADS = ["dir", "gate", "size", "tp", "sl", "hold"]


def _f(a):
    return np.ascontiguousarray(np.asarray(a, np.float32))


class _Packer:
    def __init__(self, rows):
        self.rows, self.cols, self.off, self.n = rows, [], {}, 0

    def add(self, name, arr):
        arr = _f(arr)
        assert arr.ndim == 2 and arr.shape[0] <= self.rows, (name, arr.shape)
        self.off[name] = (self.n, arr.shape[0], arr.shape[1])
        pad = np.zeros((self.rows, arr.shape[1]), np.float32)
        pad[: arr.shape[0]] = arr
        self.cols.append(pad)
        self.n += arr.shape[1]

    def blob(self):
        return np.concatenate(self.cols, axis=1)


def pack_params(params):
    p = params
    wp = _Packer(128)
    vp = _Packer(128)

    for li, lp in enumerate(p["layers"]):
        wp.add(f"L{li}.Wq", lp["Wq"])
        wp.add(f"L{li}.Wk", lp["Wk"])
        wp.add(f"L{li}.Wv", lp["Wv"])
        wp.add(f"L{li}.vg1W", lp["vg1"]["W"])
        wp.add(f"L{li}.vg2W", lp["vg2"]["W"])
        wp.add(f"L{li}.outW", lp["out"]["W"])
        wp.add(f"L{li}.ea1W", np.concatenate(
            [_f(lp["eta1"]["W"]), _f(lp["al1"]["W"])], 1))          # [128,64]
        ea2 = np.zeros((64, 2), np.float32)
        ea2[:32, 0] = _f(lp["eta2"]["W"])[:, 0]
        ea2[32:, 1] = _f(lp["al2"]["W"])[:, 0]
        wp.add(f"L{li}.ea2W", ea2)                                   # [64,2]
        vp.add(f"L{li}.vg1b", _f(lp["vg1"]["b"])[:, None])
        vp.add(f"L{li}.vg2b", _f(lp["vg2"]["b"])[:, None])
        vp.add(f"L{li}.outb", _f(lp["out"]["b"])[:, None])
        vp.add(f"L{li}.g", _f(lp["g"])[:, None])
        vp.add(f"L{li}.b", _f(lp["b"])[:, None])
        vp.add(f"L{li}.ea1b", np.concatenate(
            [_f(lp["eta1"]["b"]), _f(lp["al1"]["b"])])[:, None])    # [64,1]
        vp.add(f"L{li}.ea2b", np.array(
            [[_f(lp["eta2"]["b"])[0], _f(lp["al2"]["b"])[0]]]))      # [1,2]

    lw = _f(p["cms"]["lw"])
    w = np.exp(lw - lw.max())
    w = w / w.sum()
    Bp = np.zeros(D, np.float32)
    for i, lp in enumerate(p["cms"]["levels"]):
        wp.add(f"C{i}.l1W", lp["l1"]["W"])                          # [128,512]
        l2 = _f(lp["l2"]["W"])
        for k in range(4):
            wp.add(f"C{i}.l2W{k}", l2[k * 128:(k + 1) * 128])
        vp.add(f"C{i}.l1b", _f(lp["l1"]["b"]).reshape(4, 128).T)    # [128,4]
        vp.add(f"C{i}.l2b", _f(lp["l2"]["b"])[:, None])
        vp.add(f"C{i}.gp", (w[i] * _f(lp["g"]))[:, None])
        Bp = Bp + w[i] * _f(lp["b"])
    vp.add("C.Bp", Bp[:, None])
    vp.add("C.fg", _f(p["cms"]["fg"])[:, None])
    vp.add("C.fb", _f(p["cms"]["fb"])[:, None])

    rp = p["reg"]
    rd1 = _f(rp["rd1"]["W"])
    wp.add("R.rd1a", rd1[:128])
    wp.add("R.rd1b", rd1[128:])
    vp.add("R.rd1bias", _f(rp["rd1"]["b"])[:, None])
    wp.add("R.rd2W", rp["rd2"]["W"])                                # [128,3]
    vp.add("R.rd2b", _f(rp["rd2"]["b"])[:, None])                   # [3,1]
    g1W = _f(rp["g1"]["W"]) * _f(rp["gln_g"])[:, None]              # [387,64]
    b1p = _f(rp["gln_b"]) @ _f(rp["g1"]["W"]) + _f(rp["g1"]["b"])   # [64]
    wp.add("R.g1a", g1W[:128])
    wp.add("R.g1b", g1W[128:256])
    wp.add("R.g1c", g1W[256:384])
    wp.add("R.g1d", g1W[384:])                                      # [3,64]
    vp.add("R.b1p", b1p[:, None])
    vp.add("R.nw1s", -g1W.sum(0)[:, None])                          # [64,1]
    wp.add("R.g2W", rp["g2"]["W"])                                  # [64,1]
    vp.add("R.g2b", _f(rp["g2"]["b"])[:, None])                     # [1,1]
    for nm in ["q", "k", "v"]:
        wp.add(f"R.{nm}W", rp[nm]["W"])
        vp.add(f"R.{nm}b", _f(rp[nm]["b"])[:, None])
    vp.add("R.ng", _f(rp["ng"])[:, None])
    vp.add("R.nb", _f(rp["nb"])[:, None])

    wp.add("trunkW", p["trunk"]["W"])                               # [128,64]
    vp.add("trunkb", _f(p["trunk"]["b"])[:, None])                  # [64,1]
    wp.add("headsl1", np.concatenate(
        [_f(p["heads"][h]["l1"]["W"]) for h in _HEADS], 1))         # [64,192]
    vp.add("headsl1b", np.concatenate(
        [_f(p["heads"][h]["l1"]["b"]) for h in _HEADS]).reshape(2, 96).T)
    hl2 = np.zeros((96, 6), np.float32)
    for hd, h in enumerate(_HEADS):
        idx = hd % 3
        hl2[idx * 32:(idx + 1) * 32, hd] = _f(p["heads"][h]["l2"]["W"])[:, 0]
    wp.add("headsl2", hl2)                                          # [96,6]
    vp.add("headsl2b", np.array(
        [[_f(p["heads"][h]["l2"]["b"])[0] for h in _HEADS]]))        # [1,6]

    win = np.zeros((128, D), np.float32)
    win[:FIN] = _f(p["in"]["W"])
    wp.add("win", win)
    vp.add("inb", _f(p["in"]["b"])[:, None])
    vp.add("ing", _f(p["in_g"])[:, None])
    vp.add("inbeta", _f(p["in_beta"])[:, None])

    pe = _f(p["pos_enc"])[0, :S, 0, :]                              # [64,128]
    pos = np.repeat(pe, N, axis=0).T                                # [128,256]

    blobs = {
        "w128": np.ascontiguousarray(wp.blob()),
        "vecs": np.ascontiguousarray(vp.blob()),
        "pos": np.ascontiguousarray(pos),
    }
    return blobs, wp.off, vp.off


# ------------------------------------------------------------------
def _install_tile_patch():
    """Workaround: this walrus build rejects >1 sem wait on the TileContext
    exit Drain ('Too many sync wait commands'). Split the waits across
    preceding sync-engine nops, one wait each."""
    from concourse.vector_clock import ScopedClock

    def _patched_drain_and_barrier(self, tick_clock, wait_clock):
        nc = self.nc
        nops = [nc.sync.nop(nofuse=True, hint=f"drain_wait_split_{i}")
                for i in range(27)]
        drain_inst = nc.sync.drain()
        wait_clock.add_sem_waits(
            drain_inst.ins, ScopedClock({None: tick_clock.global_clock})
        )
        inst = drain_inst.ins
        si = inst.sync_info
        if si is not None and si.on_wait and len(si.on_wait) > 1:
            waits = list(si.on_wait)
            si.on_wait = waits[:1]
            rest = waits[1:]
            for i, nop in enumerate(nops):
                if i >= len(rest):
                    break
                nsi = nop.ins.sync_info
                if nsi is None:
                    nop.ins.sync_info = mybir.SyncInfo(on_wait=[rest[i]],
                                                       on_update=[])
                else:
                    nsi.on_wait = [rest[i]]
        nc.all_engine_barrier()
        assert self.sems is not None
        popped = nc._tile_sem_poison_stack.pop()
        assert popped is self._sem_poison
        nc.clear_and_free_semaphores(list(self.sems.allocated().values()))
        nc.all_engine_barrier()

    tile.TileContext._drain_and_barrier = _patched_drain_and_barrier


def _split_multi_waits(nc):
    """This walrus build rejects >1 sem wait on ANY instruction
    ('Too many sync wait commands'). Move extra waits onto same-engine
    NoOps inserted immediately before the instruction."""
    for f in nc.m.functions:
        for bb in f.blocks:
            out = []
            for inst in list(bb.instructions):
                si = inst.sync_info
                if si is not None and si.on_wait and len(si.on_wait) > 1:
                    waits = list(si.on_wait)
                    si.on_wait = waits[:1]
                    for j, w in enumerate(waits[1:]):
                        nop = mybir.InstNoOp(
                            name=f"{inst.name}-ws{j}", ins=[], outs=[])
                        nop.engine = inst.engine
                        nop.sync_info = mybir.SyncInfo(on_wait=[w],
                                                       on_update=[])
                        out.append(nop)
                out.append(inst)
            bb.instructions = out


def build_module(woff, voff, wcols, vcols):
    _install_tile_patch()

    nc = bass.Bass(target_bir_lowering=False, trn_type="TRN2")
    dr = dict(
        xT=nc.dram_tensor("xT", [FIN, T], F32, kind="ExternalInput"),
        w128=nc.dram_tensor("w128", [128, wcols], F32, kind="ExternalInput"),
        vecs=nc.dram_tensor("vecs", [128, vcols], F32, kind="ExternalInput"),
        pos=nc.dram_tensor("pos", [128, 256], F32, kind="ExternalInput"),
        out=nc.dram_tensor("out", [6, R], F32, kind="ExternalOutput"),
    )
    with tile.TileContext(nc) as tc:
        _emit(nc, tc, woff, voff, wcols, vcols, dr)
    _split_multi_waits(nc)
    return nc


def _emit(nc, tc, woff, voff, wcols, vcols, dr):
    import contextlib
    r32 = F32R
    ctx = contextlib.ExitStack()
    with ctx:
        const = ctx.enter_context(tc.tile_pool(name="const", bufs=1))
        persist = ctx.enter_context(tc.tile_pool(name="persist", bufs=1))
        act = ctx.enter_context(tc.tile_pool(name="act", bufs=1))
        big = ctx.enter_context(tc.tile_pool(name="bigsb", bufs=1))
        work = ctx.enter_context(tc.tile_pool(name="work", bufs=2))
        small = ctx.enter_context(tc.tile_pool(name="small", bufs=2))
        ppb = ctx.enter_context(tc.tile_pool(name="ppb", bufs=2, space="PSUM"))
        pps = ctx.enter_context(tc.tile_pool(name="pps", bufs=1, space="PSUM"))
        ppr = ctx.enter_context(tc.tile_pool(name="ppr", bufs=3, space="PSUM"))
        ppr2 = ctx.enter_context(tc.tile_pool(name="ppr2", bufs=1,
                                              space="PSUM"))

        # ---- params / input ----
        w128_sb = persist.tile([128, wcols], F32)
        nc.sync.dma_start(out=w128_sb, in_=dr["w128"][:, :])
        vecs_sb = persist.tile([128, vcols], F32)
        nc.sync.dma_start(out=vecs_sb, in_=dr["vecs"][:, :])
        pos_sb = persist.tile([128, 256], F32)
        nc.sync.dma_start(out=pos_sb, in_=dr["pos"][:, :])
        xT_sb = act.tile([FIN, T], F32, tag="ob", name="xT_sb")
        nc.sync.dma_start(out=xT_sb, in_=dr["xT"][:, :])

        def W(name):
            off, rows, cols = woff[name]
            return w128_sb[:rows, off:off + cols]

        def V(name):
            off, rows, cols = voff[name]
            return vecs_sb[:rows, off:off + cols]

        # ---- constants ----
        ones128 = const.tile([128, 128], F32)
        nc.vector.memset(ones128, 1.0)
        identb = const.tile([128, 128], BF16)
        make_identity(nc, identb)
        identf = const.tile([128, 128], F32)
        make_identity(nc, identf)
        eps_t = const.tile([128, 1], F32)
        nc.vector.memset(eps_t, 1e-5)
        zerov = const.tile([128, 1], F32)
        nc.vector.memset(zerov, 0.0)
        mask_low_neg = const.tile([64, 64], F32)
        nc.vector.memset(mask_low_neg, -1.0)
        nc.gpsimd.affine_select(out=mask_low_neg, in_=mask_low_neg,
                                pattern=[[-1, 64]], base=-1,
                                channel_multiplier=1,
                                compare_op=ALU.is_ge, fill=0.0)
        mask_up2 = const.tile([64, 128], F32)
        nc.vector.memset(mask_up2[:, 0:64], -1.0)
        nc.vector.memset(mask_up2[:, 64:128], 1.0)
        for hh in (0, 1):
            nc.gpsimd.affine_select(out=mask_up2[:, hh * 64:hh * 64 + 64],
                                    in_=mask_up2[:, hh * 64:hh * 64 + 64],
                                    pattern=[[1, 64]], base=-1,
                                    channel_multiplier=-1,
                                    compare_op=ALU.is_ge, fill=0.0)

        # ---- helpers ----
        def mm(psum_ap, lhsT_ap, rhs_ap, start=True, stop=True):
            nc.tensor.matmul(psum_ap, lhsT_ap, rhs_ap, start=start, stop=stop)

        def colsum1(dst_ps, src_ap, start=True, stop=True):
            p = src_ap.partition_size()
            nc.tensor.matmul(dst_ps, ones128[:p, 0:1],
                             src_ap, start=start, stop=stop)

        def bc1(dst_ps, src_ap, p=128):
            nc.tensor.matmul(dst_ps, ones128[0:1, :p],
                             src_ap, start=True, stop=True)

        def bcast(in1_ap, reps):
            """AP view broadcasting [p, f] -> [p, f, reps] via 0-step."""
            return bass.AP(tensor=in1_ap.tensor, offset=in1_ap.offset,
                           ap=list(in1_ap.ap) + [[0, reps]])

        def ln_chunk(dst_ap, src_ap, g_ap, b_ap, f=CH):
            """LN over the 128 partitions of src [128, f] (SBUF) -> dst.
            Stats via all-ones stationary matmul (sum broadcast to all
            partitions in one op)."""
            s1bc = ppb.tile([128, CH], F32, tag="big")
            nc.tensor.matmul(s1bc[:, :f], ones128, src_ap,
                             start=True, stop=True)
            xm = work.tile([128, CH], F32, tag="lnxm")
            nc.vector.scalar_tensor_tensor(
                out=xm[:, :f], in0=s1bc[:, :f], scalar=-1.0 / 128,
                in1=src_ap, op0=ALU.mult, op1=ALU.add)
            sq = work.tile([128, CH], F32, tag="lnsq")
            nc.gpsimd.tensor_mul(sq[:, :f], xm[:, :f], xm[:, :f])
            s2bc = ppb.tile([128, CH], F32, tag="big")
            nc.tensor.matmul(s2bc[:, :f], ones128, sq[:, :f],
                             start=True, stop=True)
            sd = work.tile([128, CH], F32, tag="lnsd2", bufs=1)
            nc.scalar.activation(sd[:, :f], s2bc[:, :f], AF.Sqrt,
                                 bias=eps_t, scale=1.0 / 128)
            rs = work.tile([128, CH], F32, tag="lnsq", name="lnrs2")
            nc.vector.reciprocal(rs[:, :f], sd[:, :f])
            nc.vector.tensor_mul(xm[:, :f], xm[:, :f], rs[:, :f])
            nc.vector.tensor_scalar(out=dst_ap, in0=xm[:, :f],
                                    scalar1=g_ap, scalar2=b_ap,
                                    op0=ALU.mult, op1=ALU.add)

        # ================= stage A: embed =================
        h = act.tile([128, T], F32, tag="h")
        for c in range(NCH):
            sl = slice(c * CH, (c + 1) * CH)
            pe = ppb.tile([128, CH], F32, tag="big")
            mm(pe, W("win")[:FIN], xT_sb[:FIN, sl])
            t0 = work.tile([128, CH], F32, tag="emb")
            nc.vector.tensor_scalar(out=t0, in0=pe, scalar1=V("inb"),
                                    scalar2=None, op0=ALU.add)
            ln_chunk(t0, t0, V("ing"), V("inbeta"))
            nc.scalar.activation(h[:, sl], t0, AF.Gelu)
        hv = h.rearrange("p (b t) -> p b t", t=256)
        pos_bc = bass.AP(tensor=pos_sb.tensor, offset=pos_sb.offset,
                         ap=[pos_sb.ap[0], [0, BL], pos_sb.ap[1]])
        nc.vector.tensor_tensor(out=hv, in0=hv, in1=pos_bc, op=ALU.add)

        # ================= delta layers =================
        for li in range(3):
            L = f"L{li}"
            qt = big.tile([128, T], BF16, tag="qt")
            kt = big.tile([128, T], BF16, tag="kt")
            kh = big.tile([128, T], BF16, tag="kh")
            ub = big.tile([128, T], F32, tag="ub")
            for c in range(NCH):
                sl = slice(c * CH, (c + 1) * CH)
                # eta/alpha per-token scalars
                p1 = ppb.tile([128, CH], F32, tag="big")
                mm(p1[:64], W(f"{L}.ea1W"), h[:, sl])
                silc = work.tile([64, CH], F32, tag="silc")
                nc.scalar.activation(silc, p1[:64], AF.Silu,
                                     bias=V(f"{L}.ea1b"))
                pe2 = pps.tile([1, CH], F32, tag="s1p")
                mm(pe2, W(f"{L}.ea2W")[0:32, 0:1], silc[0:32])
                pa2 = pps.tile([1, CH], F32, tag="s2p")
                mm(pa2, W(f"{L}.ea2W")[32:64, 1:2], silc[32:64])
                sg0 = work.tile([1, CH], F32, tag="sg0")
                nc.scalar.activation(sg0, pe2, AF.Sigmoid,
                                     bias=V(f"{L}.ea2b")[0:1, 0:1])
                sg1 = work.tile([1, CH], F32, tag="sg1")
                nc.scalar.activation(sg1, pa2, AF.Sigmoid,
                                     bias=V(f"{L}.ea2b")[0:1, 1:2])
                erc = work.tile([1, CH], F32, tag="erc")
                nc.vector.tensor_scalar(out=erc, in0=sg0,
                                        scalar1=0.1, scalar2=0.01,
                                        op0=ALU.mult, op1=ALU.add)
                arc = work.tile([1, CH], F32, tag="arc")
                nc.vector.tensor_scalar(out=arc, in0=sg1,
                                        scalar1=0.5, scalar2=0.5,
                                        op0=ALU.mult, op1=ALU.add)
                # decay cumprods (per 64-token sequence)
                pa = ppb.tile([128, CH], F32, tag="big")
                bc1(pa, arc)
                gamc = work.tile([128, CH], F32, tag="gamc")
                for rr in range(CH // S):
                    nc.vector.tensor_tensor_scan(
                        out=gamc[:, rr * S:rr * S + S],
                        data0=pa[:, rr * S:rr * S + S],
                        data1=ones128[:, 0:S], initial=1.0,
                        op0=ALU.mult, op1=ALU.bypass)
                gexc = work.tile([128, CH], F32, tag="gexc")
                gev = gexc.rearrange("p (r s) -> p r s", s=S)
                gamv = gamc.rearrange("p (r s) -> p r s", s=S)
                nc.vector.memset(gev[:, :, 0:1], 1.0)
                nc.gpsimd.tensor_copy(out=gev[:, :, 1:S],
                                      in_=gamv[:, :, 0:S - 1])
                cinvc = work.tile([128, CH], F32, tag="cinvc")
                nc.vector.reciprocal(cinvc, gamc)
                pb = ppb.tile([128, CH], F32, tag="big")
                bc1(pb, erc)
                ehatc = work.tile([128, CH], F32, tag="ehatc")
                nc.vector.tensor_mul(ehatc, pb, gexc)
                ebcc = work.tile([128, CH], F32, tag="ebcc")
                nc.scalar.copy(ebcc, pb)
                # q/k/v, value gate, scaled copies
                pq = ppb.tile([128, CH], F32, tag="big")
                mm(pq, W(f"{L}.Wq"), h[:, sl])
                nc.vector.tensor_mul(qt[:, sl], pq, gexc)
                pk = ppb.tile([128, CH], F32, tag="big")
                mm(pk, W(f"{L}.Wk"), h[:, sl])
                ksb = work.tile([128, CH], F32, tag="ksb")
                nc.scalar.copy(ksb, pk)
                sqk = work.tile([128, CH], F32, tag="sqk")
                nc.scalar.square(sqk, pk)
                s1p = pps.tile([1, CH], F32, tag="s1p")
                colsum1(s1p, sqk)
                sd = small.tile([1, CH], F32, tag="knsd", bufs=1)
                nc.scalar.sqrt(sd, s1p)
                nc.vector.tensor_scalar(out=sd, in0=sd, scalar1=1e-12,
                                        scalar2=None, op0=ALU.max)
                rn = small.tile([1, CH], F32, tag="knrn", bufs=1)
                nc.vector.reciprocal(rn, sd)
                prn = ppb.tile([128, CH], F32, tag="big")
                bc1(prn, rn)
                rc = work.tile([128, CH], F32, tag="rc")
                nc.vector.tensor_mul(rc, prn, cinvc)
                re = work.tile([128, CH], F32, tag="re")
                nc.vector.tensor_mul(re, prn, ehatc)
                nc.gpsimd.tensor_mul(kt[:, sl], ksb, rc)
                nc.gpsimd.tensor_mul(kh[:, sl], ksb, re)
                pv = ppb.tile([128, CH], F32, tag="big")
                mm(pv, W(f"{L}.Wv"), h[:, sl])
                vsb = work.tile([128, CH], F32, tag="vsb")
                nc.scalar.copy(vsb, pv)
                pz = ppb.tile([128, CH], F32, tag="big")
                mm(pz, W(f"{L}.vg1W"), vsb)
                zsb = work.tile([128, CH], F32, tag="zsb")
                nc.scalar.activation(zsb, pz, AF.Silu, bias=V(f"{L}.vg1b"))
                pvh = ppb.tile([128, CH], F32, tag="big")
                mm(pvh, W(f"{L}.vg2W"), zsb)
                vh = work.tile([128, CH], F32, tag="vh")
                nc.scalar.activation(vh, pvh, AF.Identity,
                                     bias=V(f"{L}.vg2b"))
                nc.vector.tensor_mul(ub[:, sl], vh, ebcc)

            # --- per-sequence closed-form scan ---
            ob = act.tile([128, T], F32, tag="ob")
            for rr in range(R):
                sl = slice(rr * S, rr * S + S)
                pA = ppr.tile([64, 128], F32, tag="pr")
                nc.tensor.matmul(pA[:, 0:64], kt[:, sl], kh[:, sl],
                                 start=True, stop=True)
                nc.tensor.matmul(pA[:, 64:128], kt[:, sl], qt[:, sl],
                                 start=True, stop=True)
                lta = small.tile([64, 128], BF16, tag="lta")
                nc.vector.tensor_mul(lta, pA, mask_up2)
                pU = ppr.tile([64, 128], F32, tag="pr")
                nc.tensor.transpose(pU, ub[:, sl], identf)
                ut = small.tile([64, 128], BF16, tag="ut")
                nc.scalar.copy(ut, pU)
                pW = ppr.tile([64, 128], F32, tag="pr")
                nc.tensor.matmul(pW, lta[:, 0:64], ut, start=True, stop=True)
                wsb = small.tile([64, 128], BF16, tag="wsb")
                nc.vector.tensor_add(wsb, ut, pW)
                pO = ppr.tile([128, 64], F32, tag="pr")
                nc.tensor.matmul(pO, wsb, lta[:, 64:128],
                                 start=True, stop=True)
                nc.scalar.copy(ob[:, sl], pO)

            # --- output proj + residual + LN (in-place into h) ---
            for c in range(NCH):
                sl = slice(c * CH, (c + 1) * CH)
                po = ppb.tile([128, CH], F32, tag="big")
                mm(po, W(f"{L}.outW"), ob[:, sl])
                zz = work.tile([128, CH], F32, tag="zz")
                nc.vector.scalar_tensor_tensor(
                    out=zz, in0=po, scalar=V(f"{L}.outb"),
                    in1=h[:, sl], op0=ALU.add, op1=ALU.add)
                ln_chunk(h[:, sl], zz, V(f"{L}.g"), V(f"{L}.b"))

        # ================= cms =================
        for c in range(NCH):
            sl = slice(c * CH, (c + 1) * CH)
            agg = work.tile([128, CH], F32, tag="agg")
            for i in range(4):
                zk = [None] * 4
                for k in range(4):
                    pz = ppb.tile([128, CH], F32, tag="big")
                    mm(pz, W(f"C{i}.l1W")[:, k * 128:(k + 1) * 128], h[:, sl])
                    zkk = work.tile([128, CH], F32, tag=f"cz{k}")
                    zk[k] = zkk
                    nc.scalar.activation(zk[k], pz, AF.Silu,
                                         bias=V(f"C{i}.l1b")[:, k:k + 1])
                p2 = ppb.tile([128, CH], F32, tag="big")
                for k in range(4):
                    mm(p2, W(f"C{i}.l2W{k}"), zk[k], start=(k == 0),
                       stop=(k == 3))
                zz = work.tile([128, CH], F32, tag="zz")
                nc.vector.scalar_tensor_tensor(
                    out=zz, in0=p2, scalar=V(f"C{i}.l2b"),
                    in1=h[:, sl], op0=ALU.add, op1=ALU.add)
                if i == 0:
                    ln_chunk(agg, zz, V(f"C{i}.gp"), V("C.Bp"))
                else:
                    yl = work.tile([128, CH], F32, tag="yl")
                    ln_chunk(yl, zz, V(f"C{i}.gp"), zerov)
                    nc.vector.tensor_add(agg, agg, yl)
            ln_chunk(h[:, sl], agg, V("C.fg"), V("C.fb"))

        # ================= regime =================
        M = R  # 32 (b, n) tokens
        hv4 = h.rearrange("p (b s n) -> p b n s", b=BL, s=S, n=N)
        state = small.tile([128, M], F32, tag="state")
        nc.vector.tensor_reduce(out=state.rearrange("p (b n) -> p b n", n=N),
                                in_=hv4[:, :, :, 61:64], axis=AX.X,
                                op=ALU.add)
        nc.scalar.mul(state, state, 1.0 / 3)
        gm = small.tile([128, BL], F32, tag="gm")
        nc.vector.tensor_reduce(out=gm,
                                in_=state.rearrange("p (b n) -> p b n", n=N),
                                axis=AX.X, op=ALU.add)
        nc.scalar.mul(gm, gm, 1.0 / 4)
        gmb = small.tile([128, M], F32, tag="gmb")
        nc.vector.tensor_copy(out=gmb.rearrange("p (b n) -> p b n", n=N),
                              in_=bcast(gm, N))
        xc = small.tile([128, M], F32, tag="xc")
        nc.vector.tensor_sub(xc, state, gmb)
        sqc = small.tile([128, M], F32, tag="sqc")
        nc.vector.tensor_mul(sqc, xc, xc)
        vr = small.tile([128, BL], F32, tag="vr")
        nc.vector.tensor_reduce(out=vr,
                                in_=sqc.rearrange("p (b n) -> p b n", n=N),
                                axis=AX.X, op=ALU.add)
        sdv = small.tile([128, BL], F32, tag="sdv")
        nc.scalar.activation(sdv, vr, AF.Sqrt, scale=1.0 / 3, bias=zerov)
        gsb = small.tile([128, M], F32, tag="gsb")
        nc.vector.tensor_copy(out=gsb.rearrange("p (b n) -> p b n", n=N),
                              in_=bcast(sdv, N))
        # rd MLP + probs
        p1 = ppr.tile([128, M], F32, tag="pr")
        mm(p1, W("R.rd1a"), state, start=True, stop=False)
        mm(p1, W("R.rd1b"), gmb, start=False, stop=True)
        s1 = small.tile([128, M], F32, tag="rs1")
        nc.scalar.activation(s1, p1, AF.Silu, bias=V("R.rd1bias"))
        p2 = ppr.tile([3, M], F32, tag="pr")
        mm(p2, W("R.rd2W"), s1)
        ex = small.tile([3, M], F32, tag="rex")
        nc.scalar.activation(ex, p2, AF.Exp, bias=V("R.rd2b"))
        psm = pps.tile([1, M], F32, tag="s1p")
        colsum1(psm[0:1], ex)
        rsm = small.tile([1, M], F32, tag="rsm")
        nc.vector.reciprocal(rsm, psm[0:1])
        pbc3 = ppr.tile([3, M], F32, tag="pr")
        bc1(pbc3, rsm, p=3)
        probs = small.tile([3, M], F32, tag="probs")
        nc.vector.tensor_mul(probs, ex, pbc3)
        # gi layernorm (387 dims), affine folded into g1 on host
        gs1 = pps.tile([1, M], F32, tag="s1p")
        colsum1(gs1[0:1], state, start=True, stop=False)
        colsum1(gs1[0:1], gmb, start=False, stop=False)
        colsum1(gs1[0:1], gsb, start=False, stop=False)
        colsum1(gs1[0:1], probs, start=False, stop=True)
        gs2 = pps.tile([1, M], F32, tag="s2p")
        sq1 = small.tile([128, M], F32, tag="rsq")
        for i, srcp in enumerate([state, gmb, gsb]):
            nc.scalar.square(sq1, srcp)
            colsum1(gs2[0:1], sq1, start=(i == 0), stop=False)
        nc.scalar.square(sq1[:3], probs)
        colsum1(gs2[0:1], sq1[:3], start=False, stop=True)
        mgi = small.tile([1, M], F32, tag="mgi")
        nc.scalar.mul(mgi, gs1[0:1], 1.0 / 387)
        msq = small.tile([1, M], F32, tag="msq")
        nc.scalar.square(msq, mgi)
        vgi = small.tile([1, M], F32, tag="vgi")
        nc.vector.scalar_tensor_tensor(out=vgi, in0=gs2[0:1],
                                       scalar=1.0 / 387, in1=msq,
                                       op0=ALU.mult, op1=ALU.subtract)
        sdg = small.tile([1, M], F32, tag="sdg")
        nc.scalar.activation(sdg, vgi, AF.Sqrt, bias=eps_t[:1])
        rsg = small.tile([1, M], F32, tag="rsg")
        nc.vector.reciprocal(rsg, sdg)
        pg1 = ppr.tile([64, M], F32, tag="pr")
        mm(pg1, W("R.g1a"), state, start=True, stop=False)
        mm(pg1, W("R.g1b"), gmb, start=False, stop=False)
        mm(pg1, W("R.g1c"), gsb, start=False, stop=False)
        mm(pg1, W("R.g1d"), probs, start=False, stop=True)
        pg1s = small.tile([64, M], F32, tag="pg1s")
        nc.vector.tensor_copy(pg1s, pg1)
        pmb = ppr2.tile([64, M], F32, tag="pr2")
        bc1(pmb, mgi, p=64)
        t1 = small.tile([64, M], F32, tag="t1")
        nc.vector.scalar_tensor_tensor(out=t1, in0=pmb, scalar=V("R.nw1s"),
                                       in1=pg1s, op0=ALU.mult, op1=ALU.add)
        prb = ppr.tile([64, M], F32, tag="pr")
        bc1(prb, rsg, p=64)
        t2 = small.tile([64, M], F32, tag="t2")
        nc.vector.tensor_mul(t2, t1, prb)
        rl = small.tile([64, M], F32, tag="rl")
        nc.scalar.activation(rl, t2, AF.Relu, bias=V("R.b1p"))
        pg2 = pps.tile([1, M], F32, tag="s1p")
        mm(pg2[0:1], W("R.g2W"), rl)
        alpha = small.tile([1, M], F32, tag="alpha")
        nc.scalar.activation(alpha, pg2[0:1], AF.Sigmoid, bias=V("R.g2b"))
        # attention over n
        qkv = {}
        for nm in ["q", "k", "v"]:
            pq = ppr.tile([128, M], F32, tag="pr")
            mm(pq, W(f"R.{nm}W"), state)
            qv = small.tile([128, M], F32, tag=f"r{nm}", name=f"r{nm}")
            qkv[nm] = qv
            nc.vector.tensor_scalar(out=qkv[nm], in0=pq,
                                    scalar1=V(f"R.{nm}b"), scalar2=None,
                                    op0=ALU.add)
        psc = ppr.tile([4, M], F32, tag="pr")
        for b in range(BL):
            sl4 = slice(b * 4, b * 4 + 4)
            nc.tensor.matmul(psc[:, sl4], qkv["q"][:, sl4],
                             qkv["k"][:, sl4], start=True, stop=True)
        mx = small.tile([4, BL], F32, tag="mx")
        nc.vector.tensor_reduce(out=mx,
                                in_=psc.rearrange("p (b n) -> p b n", n=N),
                                axis=AX.X, op=ALU.max)
        scs = small.tile([4, M], F32, tag="scs")
        nc.vector.tensor_tensor(out=scs.rearrange("p (b n) -> p b n", n=N),
                                in0=psc.rearrange("p (b n) -> p b n", n=N),
                                in1=bcast(mx, N), op=ALU.subtract)
        exs = small.tile([4, M], F32, tag="exs")
        nc.scalar.activation(exs, scs, AF.Exp, scale=RSQ, bias=zerov[:4])
        sms = small.tile([4, BL], F32, tag="sms")
        nc.vector.tensor_reduce(out=sms,
                                in_=exs.rearrange("p (b n) -> p b n", n=N),
                                axis=AX.X, op=ALU.add)
        rms = small.tile([4, BL], F32, tag="rms")
        nc.vector.reciprocal(rms, sms)
        attn = small.tile([4, M], F32, tag="attn")
        nc.vector.tensor_tensor(out=attn.rearrange("p (b n) -> p b n", n=N),
                                in0=exs.rearrange("p (b n) -> p b n", n=N),
                                in1=bcast(rms, N), op=ALU.mult)
        dif = small.tile([4, M], F32, tag="dif")
        eye_bc = bass.AP(tensor=identf.tensor, offset=identf.offset,
                         ap=[[identf.ap[0][0], 4], [0, BL],
                             [identf.ap[1][0], 4]])
        nc.vector.tensor_tensor(out=dif.rearrange("p (b n) -> p b n", n=N),
                                in0=eye_bc,
                                in1=attn.rearrange("p (b n) -> p b n", n=N),
                                op=ALU.subtract)
        pab = ppr2.tile([4, M], F32, tag="pr2")
        bc1(pab, alpha, p=4)
        mixed = small.tile([4, M], F32, tag="mixed")
        nc.vector.tensor_mul(mixed, pab, dif)
        nc.vector.tensor_add(mixed, mixed, attn)
        pmt = ppr.tile([4, M], F32, tag="pr")
        for b in range(BL):
            sl4 = slice(b * 4, b * 4 + 4)
            nc.tensor.transpose(pmt[:, sl4], mixed[:, sl4], identf[:4, :4])
        mT = small.tile([4, M], F32, tag="mT")
        nc.scalar.copy(mT, pmt)
        vt = small.tile([4, BL * 128], F32, tag="vt", bufs=1)
        for b in range(BL):
            pvt = ppr2.tile([4, 128], F32, tag="pr2")
            nc.tensor.transpose(pvt, qkv["v"][:, b * 4:b * 4 + 4], identf)
            nc.scalar.copy(vt[:, b * 128:(b + 1) * 128], pvt)
        pgo = ppr.tile([128, M], F32, tag="pr")
        for b in range(BL):
            mm(pgo[:, b * 4:b * 4 + 4], vt[:, b * 128:(b + 1) * 128],
               mT[:, b * 4:b * 4 + 4])
        zg = small.tile([128, M], F32, tag="zg")
        nc.vector.tensor_add(zg, pgo, state)
        gout = small.tile([128, M], F32, tag="gout")
        ln_chunk(gout, zg, V("R.ng"), V("R.nb"), f=M)

        # ================= heads =================
        hlast = h.rearrange("p (b s n) -> p s b n", s=S, n=N)[:, S - 1]
        comb = small.tile([128, M], F32, tag="comb")
        nc.vector.tensor_tensor(out=comb.rearrange("p (b n) -> p b n", n=N),
                                in0=hlast,
                                in1=gout.rearrange("p (b n) -> p b n", n=N),
                                op=ALU.add)
        ptr = ppr.tile([64, M], F32, tag="pr")
        mm(ptr, W("trunkW"), comb)
        tsb = small.tile([64, M], F32, tag="tsb")
        nc.scalar.activation(tsb, ptr, AF.Gelu, bias=V("trunkb"))
        hgel = small.tile([96, 2 * M], F32, tag="hgel")
        for half in range(2):
            ph = ppr.tile([96, M], F32, tag="pr")
            mm(ph, W("headsl1")[:, half * 96:(half + 1) * 96], tsb)
            nc.scalar.activation(hgel[:, half * M:(half + 1) * M], ph,
                                 AF.Gelu, bias=V("headsl1b")[:96,
                                                             half:half + 1])
        osb = small.tile([1, 6 * M], F32, tag="osb")
        haff = [None, None, None, (5.5, 0.5), (2.7, 0.3), (22.0, 2.0)]
        for hd in range(6):
            half, idx = divmod(hd, 3)
            rhs = hgel[idx * 32:(idx + 1) * 32, half * M:(half + 1) * M]
            ph2 = pps.tile([1, M], F32, tag="s1p")
            nc.tensor.matmul(
                ph2,
                W("headsl2")[idx * 32:(idx + 1) * 32, hd:hd + 1],
                rhs, start=True, stop=True)
            oslice = osb[0:1, hd * M:(hd + 1) * M]
            nc.scalar.activation(oslice, ph2,
                                 AF.Tanh if hd == 0 else AF.Sigmoid,
                                 bias=V("headsl2b")[0:1, hd:hd + 1])
            if haff[hd] is not None:
                m_, b_ = haff[hd]
                nc.vector.tensor_scalar(out=oslice, in0=oslice,
                                        scalar1=m_, scalar2=b_,
                                        op0=ALU.mult, op1=ALU.add)
        nc.sync.dma_start(out=dr["out"][:, :],
                          in_=osb.rearrange("p (hh m) -> p hh m", m=M))


# ------------------------------------------------------------------
_CACHE = {}
LAST_RESULTS = None


def kernel(x, params):
    import jax
    x = np.asarray(x, np.float32)
    params = jax.tree_util.tree_map(lambda a: np.asarray(a), params)
    blobs, woff, voff = pack_params(params)
    wcols = blobs["w128"].shape[1]
    vcols = blobs["vecs"].shape[1]

    key = (wcols, vcols)
    if key not in _CACHE:
        _CACHE[key] = build_module(woff, voff, wcols, vcols)
    nc = _CACHE[key]

    in_maps = []
    for c in range(NCORES):
        xs = x[c * BL:(c + 1) * BL]               # [BL, S, N, FIN]
        xT = np.ascontiguousarray(xs.reshape(T, FIN).T)
        in_maps.append({"xT": xT, "w128": blobs["w128"],
                        "vecs": blobs["vecs"], "pos": blobs["pos"]})

    from concourse.bass_utils import run_bass_kernel_spmd
    res = run_bass_kernel_spmd(nc, in_maps, core_ids=list(range(NCORES)))
    global LAST_RESULTS
    LAST_RESULTS = res
    outs = [res.results[c]["out"].reshape(6, BL, N) for c in range(NCORES)]
    return np.concatenate(outs, axis=1)             # [6, B, N]


# revision 24
# speedup vs baseline: 1.0275x; 1.0275x over previous
"""Trainium2 Bass kernel for nn_NestedGraphTitanV6 (gated delta-rule memory net).

Sharding: data-parallel over B across 8 cores (B_loc=8 -> 2048 tokens,
32 recurrence sequences of length 64 per core; the torch-style .view reshape
makes consecutive 64-token blocks the scan sequences, so no data movement).

Layout: feature-major activations [d=128 partitions, tokens free].

Scan: with M0=0 and eta in [0.01, 0.11], the S=64 delta-rule scan is computed
in closed form via the WY representation; (I+L)^{-1} ~= (I-L) with L
strictly-lower [64x64] (truncation validated: final output rel err ~5e-7 vs
the oracle, measured on hardware).

Dtypes: fp32 GEMMs (this walrus build rejects bitcast-fp32r operands);
bf16 for the per-sequence 64x64 chain. LayerNorm stats use an all-ones
stationary matmul so the sum arrives broadcast to all partitions in one op.
"""
import numpy as np
import ml_dtypes

import concourse.bass as bass
import concourse.mybir as mybir
import concourse.tile as tile
from concourse.masks import make_identity

F32 = mybir.dt.float32
F32R = mybir.dt.float32r
BF16 = mybir.dt.bfloat16
AF = mybir.ActivationFunctionType
ALU = mybir.AluOpType
AX = mybir.AxisListType

B, S, N, FIN, D = 64, 64, 4, 23, 128
NCORES = 8
BL = B // NCORES      # 8
T = BL * S * N        # 2048 tokens / core
R = BL * N            # 32 sequences / core
CH = 512              # token chunk
NCH = T // CH
RSQ = float(1.0 / np.sqrt(np.float32(128.0)))

_HEADS = ["dir", "gate", "size", "tp", "sl", "hold"]


def _f(a):
    return np.ascontiguousarray(np.asarray(a, np.float32))


class _Packer:
    def __init__(self, rows):
        self.rows, self.cols, self.off, self.n = rows, [], {}, 0

    def add(self, name, arr):
        arr = _f(arr)
        assert arr.ndim == 2 and arr.shape[0] <= self.rows, (name, arr.shape)
        self.off[name] = (self.n, arr.shape[0], arr.shape[1])
        pad = np.zeros((self.rows, arr.shape[1]), np.float32)
        pad[: arr.shape[0]] = arr
        self.cols.append(pad)
        self.n += arr.shape[1]

    def blob(self):
        return np.concatenate(self.cols, axis=1)


def pack_params(params):
    p = params
    wp = _Packer(128)
    wb = _Packer(128)   # heavy GEMM weights, shipped as bf16
    vp = _Packer(128)

    for li, lp in enumerate(p["layers"]):
        wb.add(f"L{li}.Wq", lp["Wq"])
        wb.add(f"L{li}.Wk", lp["Wk"])
        wb.add(f"L{li}.Wv", lp["Wv"])
        wb.add(f"L{li}.vg1W", lp["vg1"]["W"])
        wb.add(f"L{li}.vg2W", lp["vg2"]["W"])
        wb.add(f"L{li}.outW", lp["out"]["W"])
        wb.add(f"L{li}.ea1W", np.concatenate(
            [_f(lp["eta1"]["W"]), _f(lp["al1"]["W"])], 1))          # [128,64]
        ea2 = np.zeros((64, 2), np.float32)
        ea2[:32, 0] = _f(lp["eta2"]["W"])[:, 0]
        ea2[32:, 1] = _f(lp["al2"]["W"])[:, 0]
        wb.add(f"L{li}.ea2W", ea2)                                   # [64,2]
        vp.add(f"L{li}.vg1b", _f(lp["vg1"]["b"])[:, None])
        vp.add(f"L{li}.vg2b", _f(lp["vg2"]["b"])[:, None])
        vp.add(f"L{li}.outb", _f(lp["out"]["b"])[:, None])
        vp.add(f"L{li}.g", _f(lp["g"])[:, None])
        vp.add(f"L{li}.b", _f(lp["b"])[:, None])
        vp.add(f"L{li}.ea1b", np.concatenate(
            [_f(lp["eta1"]["b"]), _f(lp["al1"]["b"])])[:, None])    # [64,1]
        vp.add(f"L{li}.ea2b", np.array(
            [[_f(lp["eta2"]["b"])[0], _f(lp["al2"]["b"])[0]]]))      # [1,2]

    lw = _f(p["cms"]["lw"])
    w = np.exp(lw - lw.max())
    w = w / w.sum()
    Bp = np.zeros(D, np.float32)
    for i, lp in enumerate(p["cms"]["levels"]):
        wb.add(f"C{i}.l1W", lp["l1"]["W"])                          # [128,512]
        l2 = _f(lp["l2"]["W"])
        for k in range(4):
            wb.add(f"C{i}.l2W{k}", l2[k * 128:(k + 1) * 128])
        vp.add(f"C{i}.l1b", _f(lp["l1"]["b"]).reshape(4, 128).T)    # [128,4]
        vp.add(f"C{i}.l2b", _f(lp["l2"]["b"])[:, None])
        vp.add(f"C{i}.gp", (w[i] * _f(lp["g"]))[:, None])
        Bp = Bp + w[i] * _f(lp["b"])
    vp.add("C.Bp", Bp[:, None])
    vp.add("C.fg", _f(p["cms"]["fg"])[:, None])
    vp.add("C.fb", _f(p["cms"]["fb"])[:, None])

    rp = p["reg"]
    rd1 = _f(rp["rd1"]["W"])
    wp.add("R.rd1a", rd1[:128])
    wp.add("R.rd1b", rd1[128:])
    vp.add("R.rd1bias", _f(rp["rd1"]["b"])[:, None])
    wp.add("R.rd2W", rp["rd2"]["W"])                                # [128,3]
    vp.add("R.rd2b", _f(rp["rd2"]["b"])[:, None])                   # [3,1]
    g1W = _f(rp["g1"]["W"]) * _f(rp["gln_g"])[:, None]              # [387,64]
    b1p = _f(rp["gln_b"]) @ _f(rp["g1"]["W"]) + _f(rp["g1"]["b"])   # [64]
    wp.add("R.g1a", g1W[:128])
    wp.add("R.g1b", g1W[128:256])
    wp.add("R.g1c", g1W[256:384])
    wp.add("R.g1d", g1W[384:])                                      # [3,64]
    vp.add("R.b1p", b1p[:, None])
    vp.add("R.nw1s", -g1W.sum(0)[:, None])                          # [64,1]
    wp.add("R.g2W", rp["g2"]["W"])                                  # [64,1]
    vp.add("R.g2b", _f(rp["g2"]["b"])[:, None])                     # [1,1]
    for nm in ["q", "k", "v"]:
        wp.add(f"R.{nm}W", rp[nm]["W"])
        vp.add(f"R.{nm}b", _f(rp[nm]["b"])[:, None])
    vp.add("R.ng", _f(rp["ng"])[:, None])
    vp.add("R.nb", _f(rp["nb"])[:, None])

    wp.add("trunkW", p["trunk"]["W"])                               # [128,64]
    vp.add("trunkb", _f(p["trunk"]["b"])[:, None])                  # [64,1]
    wp.add("headsl1", np.concatenate(
        [_f(p["heads"][h]["l1"]["W"]) for h in _HEADS], 1))         # [64,192]
    vp.add("headsl1b", np.concatenate(
        [_f(p["heads"][h]["l1"]["b"]) for h in _HEADS]).reshape(2, 96).T)
    hl2 = np.zeros((96, 6), np.float32)
    for hd, h in enumerate(_HEADS):
        idx = hd % 3
        hl2[idx * 32:(idx + 1) * 32, hd] = _f(p["heads"][h]["l2"]["W"])[:, 0]
    wp.add("headsl2", hl2)                                          # [96,6]
    vp.add("headsl2b", np.array(
        [[_f(p["heads"][h]["l2"]["b"])[0] for h in _HEADS]]))        # [1,6]

    win = np.zeros((128, D), np.float32)
    win[:FIN] = _f(p["in"]["W"])
    wb.add("win", win)
    vp.add("inb", _f(p["in"]["b"])[:, None])
    vp.add("ing", _f(p["in_g"])[:, None])
    vp.add("inbeta", _f(p["in_beta"])[:, None])

    pe = _f(p["pos_enc"])[0, :S, 0, :]                              # [64,128]
    pos = np.repeat(pe, N, axis=0).T                                # [128,256]

    blobs = {
        "w128": np.ascontiguousarray(wp.blob()),
        "wb16": np.ascontiguousarray(wb.blob().astype(ml_dtypes.bfloat16)),
        "vecs": np.ascontiguousarray(vp.blob()),
        "pos": np.ascontiguousarray(pos),
    }
    return blobs, wp.off, wb.off, vp.off


# ------------------------------------------------------------------
def _install_tile_patch():
    """Workaround: this walrus build rejects >1 sem wait on the TileContext
    exit Drain ('Too many sync wait commands'). Split the waits across
    preceding sync-engine nops, one wait each."""
    from concourse.vector_clock import ScopedClock

    def _patched_drain_and_barrier(self, tick_clock, wait_clock):
        nc = self.nc
        nops = [nc.sync.nop(nofuse=True, hint=f"drain_wait_split_{i}")
                for i in range(27)]
        drain_inst = nc.sync.drain()
        wait_clock.add_sem_waits(
            drain_inst.ins, ScopedClock({None: tick_clock.global_clock})
        )
        inst = drain_inst.ins
        si = inst.sync_info
        if si is not None and si.on_wait and len(si.on_wait) > 1:
            waits = list(si.on_wait)
            si.on_wait = waits[:1]
            rest = waits[1:]
            for i, nop in enumerate(nops):
                if i >= len(rest):
                    break
                nsi = nop.ins.sync_info
                if nsi is None:
                    nop.ins.sync_info = mybir.SyncInfo(on_wait=[rest[i]],
                                                       on_update=[])
                else:
                    nsi.on_wait = [rest[i]]
        nc.all_engine_barrier()
        assert self.sems is not None
        popped = nc._tile_sem_poison_stack.pop()
        assert popped is self._sem_poison
        nc.clear_and_free_semaphores(list(self.sems.allocated().values()))
        nc.all_engine_barrier()

    tile.TileContext._drain_and_barrier = _patched_drain_and_barrier


def _split_multi_waits(nc):
    """This walrus build rejects >1 sem wait on ANY instruction
    ('Too many sync wait commands'). Move extra waits onto same-engine
    NoOps inserted immediately before the instruction."""
    for f in nc.m.functions:
        for bb in f.blocks:
            out = []
            for inst in list(bb.instructions):
                si = inst.sync_info
                if si is not None and si.on_wait and len(si.on_wait) > 1:
                    waits = list(si.on_wait)
                    si.on_wait = waits[:1]
                    for j, w in enumerate(waits[1:]):
                        nop = mybir.InstNoOp(
                            name=f"{inst.name}-ws{j}", ins=[], outs=[])
                        nop.engine = inst.engine
                        nop.sync_info = mybir.SyncInfo(on_wait=[w],
                                                       on_update=[])
                        out.append(nop)
                out.append(inst)
            bb.instructions = out


def build_module(woff, boff, voff, wcols, bcols, vcols):
    _install_tile_patch()

    nc = bass.Bass(target_bir_lowering=False, trn_type="TRN2")
    dr = dict(
        xT=nc.dram_tensor("xT", [FIN, T], BF16, kind="ExternalInput"),
        w128=nc.dram_tensor("w128", [128, wcols], F32, kind="ExternalInput"),
        wb16=nc.dram_tensor("wb16", [128, bcols], BF16, kind="ExternalInput"),
        vecs=nc.dram_tensor("vecs", [128, vcols], F32, kind="ExternalInput"),
        pos=nc.dram_tensor("pos", [128, 256], F32, kind="ExternalInput"),
        out=nc.dram_tensor("out", [6, R], F32, kind="ExternalOutput"),
    )
    with tile.TileContext(nc) as tc:
        _emit(nc, tc, woff, boff, voff, wcols, bcols, vcols, dr)
    _split_multi_waits(nc)
    return nc


def _emit(nc, tc, woff, boff, voff, wcols, bcols, vcols, dr):
    import contextlib
    r32 = F32R
    ctx = contextlib.ExitStack()
    with ctx:
        const = ctx.enter_context(tc.tile_pool(name="const", bufs=1))
        persist = ctx.enter_context(tc.tile_pool(name="persist", bufs=1))
        act = ctx.enter_context(tc.tile_pool(name="act", bufs=1))
        big = ctx.enter_context(tc.tile_pool(name="bigsb", bufs=1))
        work = ctx.enter_context(tc.tile_pool(name="work", bufs=2))
        small = ctx.enter_context(tc.tile_pool(name="small", bufs=2))
        ppb = ctx.enter_context(tc.tile_pool(name="ppb", bufs=2, space="PSUM"))
        pps = ctx.enter_context(tc.tile_pool(name="pps", bufs=1, space="PSUM"))
        ppr = ctx.enter_context(tc.tile_pool(name="ppr", bufs=3, space="PSUM"))
        ppr2 = ctx.enter_context(tc.tile_pool(name="ppr2", bufs=1,
                                              space="PSUM"))

        # ---- params / input ----
        w128_sb = persist.tile([128, wcols], F32)
        nc.sync.dma_start(out=w128_sb, in_=dr["w128"][:, :])
        wb16_sb = persist.tile([128, bcols], BF16)
        nc.sync.dma_start(out=wb16_sb, in_=dr["wb16"][:, :])
        vecs_sb = persist.tile([128, vcols], F32)
        nc.sync.dma_start(out=vecs_sb, in_=dr["vecs"][:, :])
        pos_sb = persist.tile([128, 256], F32)
        nc.sync.dma_start(out=pos_sb, in_=dr["pos"][:, :])
        xT_sb = act.tile([FIN, T], BF16, tag="obx", name="xT_sb", bufs=1)
        nc.sync.dma_start(out=xT_sb, in_=dr["xT"][:, :])

        def W(name):
            off, rows, cols = woff[name]
            return w128_sb[:rows, off:off + cols]

        def V(name):
            off, rows, cols = voff[name]
            return vecs_sb[:rows, off:off + cols]

        def Wb(name):
            off, rows, cols = boff[name]
            return wb16_sb[:rows, off:off + cols]

        # ---- constants ----
        ones128 = const.tile([128, 128], F32)
        nc.vector.memset(ones128, 1.0)
        identb = const.tile([128, 128], BF16)
        make_identity(nc, identb)
        identf = const.tile([128, 128], F32)
        make_identity(nc, identf)
        eps_t = const.tile([128, 1], F32)
        nc.vector.memset(eps_t, 1e-5)
        zerov = const.tile([128, 1], F32)
        nc.vector.memset(zerov, 0.0)
        mask_low_neg = const.tile([64, 64], F32)
        nc.vector.memset(mask_low_neg, -1.0)
        nc.gpsimd.affine_select(out=mask_low_neg, in_=mask_low_neg,
                                pattern=[[-1, 64]], base=-1,
                                channel_multiplier=1,
                                compare_op=ALU.is_ge, fill=0.0)
        mask_up2 = const.tile([64, 128], F32)
        nc.vector.memset(mask_up2[:, 0:64], -1.0)
        nc.vector.memset(mask_up2[:, 64:128], 1.0)
        for hh in (0, 1):
            nc.gpsimd.affine_select(out=mask_up2[:, hh * 64:hh * 64 + 64],
                                    in_=mask_up2[:, hh * 64:hh * 64 + 64],
                                    pattern=[[1, 64]], base=-1,
                                    channel_multiplier=-1,
                                    compare_op=ALU.is_ge, fill=0.0)

        # ---- helpers ----
        def mm(psum_ap, lhsT_ap, rhs_ap, start=True, stop=True):
            nc.tensor.matmul(psum_ap, lhsT_ap, rhs_ap, start=start, stop=stop)

        def colsum1(dst_ps, src_ap, start=True, stop=True):
            p = src_ap.partition_size()
            nc.tensor.matmul(dst_ps, ones128[:p, 0:1],
                             src_ap, start=start, stop=stop)

        def bc1(dst_ps, src_ap, p=128):
            nc.tensor.matmul(dst_ps, ones128[0:1, :p],
                             src_ap, start=True, stop=True)

        def bcast(in1_ap, reps):
            """AP view broadcasting [p, f] -> [p, f, reps] via 0-step."""
            return bass.AP(tensor=in1_ap.tensor, offset=in1_ap.offset,
                           ap=list(in1_ap.ap) + [[0, reps]])

        def ln_chunk(dst_ap, src_ap, g_ap, b_ap, f=CH):
            """LN over the 128 partitions of src [128, f] (SBUF) -> dst.
            Stats via all-ones stationary matmul (sum broadcast to all
            partitions in one op)."""
            s1bc = ppb.tile([128, CH], F32, tag="big")
            nc.tensor.matmul(s1bc[:, :f], ones128, src_ap,
                             start=True, stop=True)
            xm = work.tile([128, CH], F32, tag="lnxm")
            nc.vector.scalar_tensor_tensor(
                out=xm[:, :f], in0=s1bc[:, :f], scalar=-1.0 / 128,
                in1=src_ap, op0=ALU.mult, op1=ALU.add)
            sq = work.tile([128, CH], F32, tag="lnsq")
            nc.gpsimd.tensor_mul(sq[:, :f], xm[:, :f], xm[:, :f])
            s2bc = ppb.tile([128, CH], F32, tag="big")
            nc.tensor.matmul(s2bc[:, :f], ones128, sq[:, :f],
                             start=True, stop=True)
            sd = work.tile([128, CH], F32, tag="lnsd2", bufs=1)
            nc.scalar.activation(sd[:, :f], s2bc[:, :f], AF.Sqrt,
                                 bias=eps_t, scale=1.0 / 128)
            rs = work.tile([128, CH], F32, tag="lnsq", name="lnrs2")
            nc.vector.reciprocal(rs[:, :f], sd[:, :f])
            nc.vector.tensor_mul(xm[:, :f], xm[:, :f], rs[:, :f])
            nc.vector.tensor_scalar(out=dst_ap, in0=xm[:, :f],
                                    scalar1=g_ap, scalar2=b_ap,
                                    op0=ALU.mult, op1=ALU.add)

        # ================= stage A: embed =================
        h = act.tile([128, T], F32, tag="h")
        hb = act.tile([128, T], BF16, tag="hb")
        for c in range(NCH):
            sl = slice(c * CH, (c + 1) * CH)
            pe = ppb.tile([128, CH], F32, tag="big")
            mm(pe, Wb("win")[:FIN], xT_sb[:FIN, sl])
            t0 = work.tile([128, CH], F32, tag="emb")
            nc.vector.tensor_scalar(out=t0, in0=pe, scalar1=V("inb"),
                                    scalar2=None, op0=ALU.add)
            ln_chunk(t0, t0, V("ing"), V("inbeta"))
            nc.scalar.activation(h[:, sl], t0, AF.Gelu)
        hv = h.rearrange("p (b t) -> p b t", t=256)
        pos_bc = bass.AP(tensor=pos_sb.tensor, offset=pos_sb.offset,
                         ap=[pos_sb.ap[0], [0, BL], pos_sb.ap[1]])
        nc.vector.tensor_tensor(out=hv, in0=hv, in1=pos_bc, op=ALU.add)
        nc.gpsimd.tensor_copy(hb, h)

        # ================= delta layers =================
        for li in range(3):
            L = f"L{li}"
            qt = big.tile([128, T], BF16, tag="qt")
            kt = big.tile([128, T], BF16, tag="kt")
            kh = big.tile([128, T], BF16, tag="kh")
            ub = big.tile([128, T], F32, tag="ub")
            for c in range(NCH):
                sl = slice(c * CH, (c + 1) * CH)
                # eta/alpha per-token scalars
                p1 = ppb.tile([128, CH], F32, tag="big")
                mm(p1[:64], Wb(f"{L}.ea1W"), hb[:, sl])
                silc = work.tile([64, CH], BF16, tag="silc")
                nc.scalar.activation(silc, p1[:64], AF.Silu,
                                     bias=V(f"{L}.ea1b"))
                pe2 = pps.tile([1, CH], F32, tag="s1p")
                mm(pe2, Wb(f"{L}.ea2W")[0:32, 0:1], silc[0:32])
                pa2 = pps.tile([1, CH], F32, tag="s2p")
                mm(pa2, Wb(f"{L}.ea2W")[32:64, 1:2], silc[32:64])
                sg0 = work.tile([1, CH], F32, tag="sg0")
                nc.scalar.activation(sg0, pe2, AF.Sigmoid,
                                     bias=V(f"{L}.ea2b")[0:1, 0:1])
                sg1 = work.tile([1, CH], F32, tag="sg1")
                nc.scalar.activation(sg1, pa2, AF.Sigmoid,
                                     bias=V(f"{L}.ea2b")[0:1, 1:2])
                erc = work.tile([1, CH], F32, tag="erc")
                nc.vector.tensor_scalar(out=erc, in0=sg0,
                                        scalar1=0.1, scalar2=0.01,
                                        op0=ALU.mult, op1=ALU.add)
                arc = work.tile([1, CH], F32, tag="arc")
                nc.vector.tensor_scalar(out=arc, in0=sg1,
                                        scalar1=0.5, scalar2=0.5,
                                        op0=ALU.mult, op1=ALU.add)
                # decay cumprods (per 64-token sequence)
                pa = ppb.tile([128, CH], F32, tag="big")
                bc1(pa, arc)
                gamc = work.tile([128, CH], F32, tag="gamc")
                for rr in range(CH // S):
                    nc.vector.tensor_tensor_scan(
                        out=gamc[:, rr * S:rr * S + S],
                        data0=pa[:, rr * S:rr * S + S],
                        data1=ones128[:, 0:S], initial=1.0,
                        op0=ALU.mult, op1=ALU.bypass)
                gexc = work.tile([128, CH], F32, tag="gexc")
                gev = gexc.rearrange("p (r s) -> p r s", s=S)
                gamv = gamc.rearrange("p (r s) -> p r s", s=S)
                nc.vector.memset(gev[:, :, 0:1], 1.0)
                nc.gpsimd.tensor_copy(out=gev[:, :, 1:S],
                                      in_=gamv[:, :, 0:S - 1])
                cinvc = work.tile([128, CH], F32, tag="cinvc")
                nc.vector.reciprocal(cinvc, gamc)
                pb = ppb.tile([128, CH], F32, tag="big")
                bc1(pb, erc)
                ehatc = work.tile([128, CH], F32, tag="ehatc")
                nc.vector.tensor_mul(ehatc, pb, gexc)
                ebcc = work.tile([128, CH], F32, tag="ebcc")
                nc.scalar.copy(ebcc, pb)
                # q/k/v, value gate, scaled copies
                pq = ppb.tile([128, CH], F32, tag="big")
                mm(pq, Wb(f"{L}.Wq"), hb[:, sl])
                nc.vector.tensor_mul(qt[:, sl], pq, gexc)
                pk = ppb.tile([128, CH], F32, tag="big")
                mm(pk, Wb(f"{L}.Wk"), hb[:, sl])
                ksb = work.tile([128, CH], F32, tag="ksb")
                nc.scalar.copy(ksb, pk)
                sqk = work.tile([128, CH], F32, tag="sqk")
                nc.scalar.square(sqk, pk)
                s1p = pps.tile([1, CH], F32, tag="s1p")
                colsum1(s1p, sqk)
                sd = small.tile([1, CH], F32, tag="knsd", bufs=1)
                nc.scalar.sqrt(sd, s1p)
                nc.vector.tensor_scalar(out=sd, in0=sd, scalar1=1e-12,
                                        scalar2=None, op0=ALU.max)
                rn = small.tile([1, CH], F32, tag="knrn", bufs=1)
                nc.vector.reciprocal(rn, sd)
                prn = ppb.tile([128, CH], F32, tag="big")
                bc1(prn, rn)
                rc = work.tile([128, CH], F32, tag="rc")
                nc.vector.tensor_mul(rc, prn, cinvc)
                re = work.tile([128, CH], F32, tag="re")
                nc.vector.tensor_mul(re, prn, ehatc)
                nc.gpsimd.tensor_mul(kt[:, sl], ksb, rc)
                nc.gpsimd.tensor_mul(kh[:, sl], ksb, re)
                pv = ppb.tile([128, CH], F32, tag="big")
                mm(pv, Wb(f"{L}.Wv"), hb[:, sl])
                vsb = work.tile([128, CH], BF16, tag="vsb")
                nc.scalar.copy(vsb, pv)
                pz = ppb.tile([128, CH], F32, tag="big")
                mm(pz, Wb(f"{L}.vg1W"), vsb)
                zsb = work.tile([128, CH], BF16, tag="zsb")
                nc.scalar.activation(zsb, pz, AF.Silu, bias=V(f"{L}.vg1b"))
                pvh = ppb.tile([128, CH], F32, tag="big")
                mm(pvh, Wb(f"{L}.vg2W"), zsb)
                vh = work.tile([128, CH], F32, tag="vh")
                nc.scalar.activation(vh, pvh, AF.Identity,
                                     bias=V(f"{L}.vg2b"))
                nc.vector.tensor_mul(ub[:, sl], vh, ebcc)

            # --- per-sequence closed-form scan ---
            ob = act.tile([128, T], BF16, tag="ob")
            for rr in range(R):
                sl = slice(rr * S, rr * S + S)
                pA = ppr.tile([64, 128], F32, tag="pr")
                nc.tensor.matmul(pA[:, 0:64], kt[:, sl], kh[:, sl],
                                 start=True, stop=True)
                nc.tensor.matmul(pA[:, 64:128], kt[:, sl], qt[:, sl],
                                 start=True, stop=True)
                lta = small.tile([64, 128], BF16, tag="lta")
                nc.vector.tensor_mul(lta, pA, mask_up2)
                pU = ppr.tile([64, 128], F32, tag="pr")
                nc.tensor.transpose(pU, ub[:, sl], identf)
                ut = small.tile([64, 128], BF16, tag="ut")
                nc.scalar.copy(ut, pU)
                pW = ppr.tile([64, 128], F32, tag="pr")
                nc.tensor.matmul(pW, lta[:, 0:64], ut, start=True, stop=True)
                wsb = small.tile([64, 128], BF16, tag="wsb")
                nc.vector.tensor_add(wsb, ut, pW)
                pO = ppr.tile([128, 64], F32, tag="pr")
                nc.tensor.matmul(pO, wsb, lta[:, 64:128],
                                 start=True, stop=True)
                nc.scalar.copy(ob[:, sl], pO)

            # --- output proj + residual + LN (in-place into h) ---
            for c in range(NCH):
                sl = slice(c * CH, (c + 1) * CH)
                po = ppb.tile([128, CH], F32, tag="big")
                mm(po, Wb(f"{L}.outW"), ob[:, sl])
                zz = work.tile([128, CH], F32, tag="zz")
                nc.vector.scalar_tensor_tensor(
                    out=zz, in0=po, scalar=V(f"{L}.outb"),
                    in1=h[:, sl], op0=ALU.add, op1=ALU.add)
                ln_chunk(h[:, sl], zz, V(f"{L}.g"), V(f"{L}.b"))
                nc.gpsimd.tensor_copy(hb[:, sl], h[:, sl])

        # ================= cms =================
        for c in range(NCH):
            sl = slice(c * CH, (c + 1) * CH)
            agg = work.tile([128, CH], F32, tag="agg")
            for i in range(4):
                zk = [None] * 4
                for k in range(4):
                    pz = ppb.tile([128, CH], F32, tag="big")
                    mm(pz, Wb(f"C{i}.l1W")[:, k * 128:(k + 1) * 128], hb[:, sl])
                    zkk = work.tile([128, CH], BF16, tag=f"cz{k}")
                    zk[k] = zkk
                    nc.scalar.activation(zk[k], pz, AF.Silu,
                                         bias=V(f"C{i}.l1b")[:, k:k + 1])
                p2 = ppb.tile([128, CH], F32, tag="big")
                for k in range(4):
                    mm(p2, Wb(f"C{i}.l2W{k}"), zk[k], start=(k == 0),
                       stop=(k == 3))
                zz = work.tile([128, CH], F32, tag="zz")
                nc.vector.scalar_tensor_tensor(
                    out=zz, in0=p2, scalar=V(f"C{i}.l2b"),
                    in1=h[:, sl], op0=ALU.add, op1=ALU.add)
                if i == 0:
                    ln_chunk(agg, zz, V(f"C{i}.gp"), V("C.Bp"))
                else:
                    yl = work.tile([128, CH], F32, tag="yl")
                    ln_chunk(yl, zz, V(f"C{i}.gp"), zerov)
                    nc.vector.tensor_add(agg, agg, yl)
            ln_chunk(h[:, sl], agg, V("C.fg"), V("C.fb"))

        # ================= regime =================
        M = R  # 32 (b, n) tokens
        hv4 = h.rearrange("p (b s n) -> p b n s", b=BL, s=S, n=N)
        state = small.tile([128, M], F32, tag="state")
        nc.vector.tensor_reduce(out=state.rearrange("p (b n) -> p b n", n=N),
                                in_=hv4[:, :, :, 61:64], axis=AX.X,
                                op=ALU.add)
        nc.scalar.mul(state, state, 1.0 / 3)
        gm = small.tile([128, BL], F32, tag="gm")
        nc.vector.tensor_reduce(out=gm,
                                in_=state.rearrange("p (b n) -> p b n", n=N),
                                axis=AX.X, op=ALU.add)
        nc.scalar.mul(gm, gm, 1.0 / 4)
        gmb = small.tile([128, M], F32, tag="gmb")
        nc.vector.tensor_copy(out=gmb.rearrange("p (b n) -> p b n", n=N),
                              in_=bcast(gm, N))
        xc = small.tile([128, M], F32, tag="xc")
        nc.vector.tensor_sub(xc, state, gmb)
        sqc = small.tile([128, M], F32, tag="sqc")
        nc.vector.tensor_mul(sqc, xc, xc)
        vr = small.tile([128, BL], F32, tag="vr")
        nc.vector.tensor_reduce(out=vr,
                                in_=sqc.rearrange("p (b n) -> p b n", n=N),
                                axis=AX.X, op=ALU.add)
        sdv = small.tile([128, BL], F32, tag="sdv")
        nc.scalar.activation(sdv, vr, AF.Sqrt, scale=1.0 / 3, bias=zerov)
        gsb = small.tile([128, M], F32, tag="gsb")
        nc.vector.tensor_copy(out=gsb.rearrange("p (b n) -> p b n", n=N),
                              in_=bcast(sdv, N))
        # rd MLP + probs
        p1 = ppr.tile([128, M], F32, tag="pr")
        mm(p1, W("R.rd1a"), state, start=True, stop=False)
        mm(p1, W("R.rd1b"), gmb, start=False, stop=True)
        s1 = small.tile([128, M], F32, tag="rs1")
        nc.scalar.activation(s1, p1, AF.Silu, bias=V("R.rd1bias"))
        p2 = ppr.tile([3, M], F32, tag="pr")
        mm(p2, W("R.rd2W"), s1)
        ex = small.tile([3, M], F32, tag="rex")
        nc.scalar.activation(ex, p2, AF.Exp, bias=V("R.rd2b"))
        psm = pps.tile([1, M], F32, tag="s1p")
        colsum1(psm[0:1], ex)
        rsm = small.tile([1, M], F32, tag="rsm")
        nc.vector.reciprocal(rsm, psm[0:1])
        pbc3 = ppr.tile([3, M], F32, tag="pr")
        bc1(pbc3, rsm, p=3)
        probs = small.tile([3, M], F32, tag="probs")
        nc.vector.tensor_mul(probs, ex, pbc3)
        # gi layernorm (387 dims), affine folded into g1 on host
        gs1 = pps.tile([1, M], F32, tag="s1p")
        colsum1(gs1[0:1], state, start=True, stop=False)
        colsum1(gs1[0:1], gmb, start=False, stop=False)
        colsum1(gs1[0:1], gsb, start=False, stop=False)
        colsum1(gs1[0:1], probs, start=False, stop=True)
        gs2 = pps.tile([1, M], F32, tag="s2p")
        sq1 = small.tile([128, M], F32, tag="rsq")
        for i, srcp in enumerate([state, gmb, gsb]):
            nc.scalar.square(sq1, srcp)
            colsum1(gs2[0:1], sq1, start=(i == 0), stop=False)
        nc.scalar.square(sq1[:3], probs)
        colsum1(gs2[0:1], sq1[:3], start=False, stop=True)
        mgi = small.tile([1, M], F32, tag="mgi")
        nc.scalar.mul(mgi, gs1[0:1], 1.0 / 387)
        msq = small.tile([1, M], F32, tag="msq")
        nc.scalar.square(msq, mgi)
        vgi = small.tile([1, M], F32, tag="vgi")
        nc.vector.scalar_tensor_tensor(out=vgi, in0=gs2[0:1],
                                       scalar=1.0 / 387, in1=msq,
                                       op0=ALU.mult, op1=ALU.subtract)
        sdg = small.tile([1, M], F32, tag="sdg")
        nc.scalar.activation(sdg, vgi, AF.Sqrt, bias=eps_t[:1])
        rsg = small.tile([1, M], F32, tag="rsg")
        nc.vector.reciprocal(rsg, sdg)
        pg1 = ppr.tile([64, M], F32, tag="pr")
        mm(pg1, W("R.g1a"), state, start=True, stop=False)
        mm(pg1, W("R.g1b"), gmb, start=False, stop=False)
        mm(pg1, W("R.g1c"), gsb, start=False, stop=False)
        mm(pg1, W("R.g1d"), probs, start=False, stop=True)
        pg1s = small.tile([64, M], F32, tag="pg1s")
        nc.vector.tensor_copy(pg1s, pg1)
        pmb = ppr2.tile([64, M], F32, tag="pr2")
        bc1(pmb, mgi, p=64)
        t1 = small.tile([64, M], F32, tag="t1")
        nc.vector.scalar_tensor_tensor(out=t1, in0=pmb, scalar=V("R.nw1s"),
                                       in1=pg1s, op0=ALU.mult, op1=ALU.add)
        prb = ppr.tile([64, M], F32, tag="pr")
        bc1(prb, rsg, p=64)
        t2 = small.tile([64, M], F32, tag="t2")
        nc.vector.tensor_mul(t2, t1, prb)
        rl = small.tile([64, M], F32, tag="rl")
        nc.scalar.activation(rl, t2, AF.Relu, bias=V("R.b1p"))
        pg2 = pps.tile([1, M], F32, tag="s1p")
        mm(pg2[0:1], W("R.g2W"), rl)
        alpha = small.tile([1, M], F32, tag="alpha")
        nc.scalar.activation(alpha, pg2[0:1], AF.Sigmoid, bias=V("R.g2b"))
        # attention over n
        qkv = {}
        for nm in ["q", "k", "v"]:
            pq = ppr.tile([128, M], F32, tag="pr")
            mm(pq, W(f"R.{nm}W"), state)
            qv = small.tile([128, M], F32, tag=f"r{nm}", name=f"r{nm}")
            qkv[nm] = qv
            nc.vector.tensor_scalar(out=qkv[nm], in0=pq,
                                    scalar1=V(f"R.{nm}b"), scalar2=None,
                                    op0=ALU.add)
        psc = ppr.tile([4, M], F32, tag="pr")
        for b in range(BL):
            sl4 = slice(b * 4, b * 4 + 4)
            nc.tensor.matmul(psc[:, sl4], qkv["q"][:, sl4],
                             qkv["k"][:, sl4], start=True, stop=True)
        mx = small.tile([4, BL], F32, tag="mx")
        nc.vector.tensor_reduce(out=mx,
                                in_=psc.rearrange("p (b n) -> p b n", n=N),
                                axis=AX.X, op=ALU.max)
        scs = small.tile([4, M], F32, tag="scs")
        nc.vector.tensor_tensor(out=scs.rearrange("p (b n) -> p b n", n=N),
                                in0=psc.rearrange("p (b n) -> p b n", n=N),
                                in1=bcast(mx, N), op=ALU.subtract)
        exs = small.tile([4, M], F32, tag="exs")
        nc.scalar.activation(exs, scs, AF.Exp, scale=RSQ, bias=zerov[:4])
        sms = small.tile([4, BL], F32, tag="sms")
        nc.vector.tensor_reduce(out=sms,
                                in_=exs.rearrange("p (b n) -> p b n", n=N),
                                axis=AX.X, op=ALU.add)
        rms = small.tile([4, BL], F32, tag="rms")
        nc.vector.reciprocal(rms, sms)
        attn = small.tile([4, M], F32, tag="attn")
        nc.vector.tensor_tensor(out=attn.rearrange("p (b n) -> p b n", n=N),
                                in0=exs.rearrange("p (b n) -> p b n", n=N),
                                in1=bcast(rms, N), op=ALU.mult)
        dif = small.tile([4, M], F32, tag="dif")
        eye_bc = bass.AP(tensor=identf.tensor, offset=identf.offset,
                         ap=[[identf.ap[0][0], 4], [0, BL],
                             [identf.ap[1][0], 4]])
        nc.vector.tensor_tensor(out=dif.rearrange("p (b n) -> p b n", n=N),
                                in0=eye_bc,
                                in1=attn.rearrange("p (b n) -> p b n", n=N),
                                op=ALU.subtract)
        pab = ppr2.tile([4, M], F32, tag="pr2")
        bc1(pab, alpha, p=4)
        mixed = small.tile([4, M], F32, tag="mixed")
        nc.vector.tensor_mul(mixed, pab, dif)
        nc.vector.tensor_add(mixed, mixed, attn)
        pmt = ppr.tile([4, M], F32, tag="pr")
        for b in range(BL):
            sl4 = slice(b * 4, b * 4 + 4)
            nc.tensor.transpose(pmt[:, sl4], mixed[:, sl4], identf[:4, :4])
        mT = small.tile([4, M], F32, tag="mT")
        nc.scalar.copy(mT, pmt)
        vt = small.tile([4, BL * 128], F32, tag="vt", bufs=1)
        for b in range(BL):
            pvt = ppr2.tile([4, 128], F32, tag="pr2")
            nc.tensor.transpose(pvt, qkv["v"][:, b * 4:b * 4 + 4], identf)
            nc.scalar.copy(vt[:, b * 128:(b + 1) * 128], pvt)
        pgo = ppr.tile([128, M], F32, tag="pr")
        for b in range(BL):
            mm(pgo[:, b * 4:b * 4 + 4], vt[:, b * 128:(b + 1) * 128],
               mT[:, b * 4:b * 4 + 4])
        zg = small.tile([128, M], F32, tag="zg")
        nc.vector.tensor_add(zg, pgo, state)
        gout = small.tile([128, M], F32, tag="gout")
        ln_chunk(gout, zg, V("R.ng"), V("R.nb"), f=M)

        # ================= heads =================
        hlast = h.rearrange("p (b s n) -> p s b n", s=S, n=N)[:, S - 1]
        comb = small.tile([128, M], F32, tag="comb")
        nc.vector.tensor_tensor(out=comb.rearrange("p (b n) -> p b n", n=N),
                                in0=hlast,
                                in1=gout.rearrange("p (b n) -> p b n", n=N),
                                op=ALU.add)
        ptr = ppr.tile([64, M], F32, tag="pr")
        mm(ptr, W("trunkW"), comb)
        tsb = small.tile([64, M], F32, tag="tsb")
        nc.scalar.activation(tsb, ptr, AF.Gelu, bias=V("trunkb"))
        hgel = small.tile([96, 2 * M], F32, tag="hgel")
        for half in range(2):
            ph = ppr.tile([96, M], F32, tag="pr")
            mm(ph, W("headsl1")[:, half * 96:(half + 1) * 96], tsb)
            nc.scalar.activation(hgel[:, half * M:(half + 1) * M], ph,
                                 AF.Gelu, bias=V("headsl1b")[:96,
                                                             half:half + 1])
        osb = small.tile([1, 6 * M], F32, tag="osb")
        haff = [None, None, None, (5.5, 0.5), (2.7, 0.3), (22.0, 2.0)]
        for hd in range(6):
            half, idx = divmod(hd, 3)
            rhs = hgel[idx * 32:(idx + 1) * 32, half * M:(half + 1) * M]
            ph2 = pps.tile([1, M], F32, tag="s1p")
            nc.tensor.matmul(
                ph2,
                W("headsl2")[idx * 32:(idx + 1) * 32, hd:hd + 1],
                rhs, start=True, stop=True)
            oslice = osb[0:1, hd * M:(hd + 1) * M]
            nc.scalar.activation(oslice, ph2,
                                 AF.Tanh if hd == 0 else AF.Sigmoid,
                                 bias=V("headsl2b")[0:1, hd:hd + 1])
            if haff[hd] is not None:
                m_, b_ = haff[hd]
                nc.vector.tensor_scalar(out=oslice, in0=oslice,
                                        scalar1=m_, scalar2=b_,
                                        op0=ALU.mult, op1=ALU.add)
        nc.sync.dma_start(out=dr["out"][:, :],
                          in_=osb.rearrange("p (hh m) -> p hh m", m=M))


# ------------------------------------------------------------------
_CACHE = {}
LAST_RESULTS = None


def kernel(x, params):
    import jax
    x = np.asarray(x, np.float32)
    params = jax.tree_util.tree_map(lambda a: np.asarray(a), params)
    blobs, woff, boff, voff = pack_params(params)
    wcols = blobs["w128"].shape[1]
    bcols = blobs["wb16"].shape[1]
    vcols = blobs["vecs"].shape[1]

    key = (wcols, bcols, vcols)
    if key not in _CACHE:
        _CACHE[key] = build_module(woff, boff, voff, wcols, bcols, vcols)
    nc = _CACHE[key]

    in_maps = []
    for c in range(NCORES):
        xs = x[c * BL:(c + 1) * BL]               # [BL, S, N, FIN]
        xT = np.ascontiguousarray(
            xs.reshape(T, FIN).T.astype(ml_dtypes.bfloat16))
        in_maps.append({"xT": xT, "w128": blobs["w128"],
                        "wb16": blobs["wb16"], "vecs": blobs["vecs"],
                        "pos": blobs["pos"]})

    from concourse.bass_utils import run_bass_kernel_spmd
    res = run_bass_kernel_spmd(nc, in_maps, core_ids=list(range(NCORES)))
    global LAST_RESULTS
    LAST_RESULTS = res
    outs = [res.results[c]["out"].reshape(6, BL, N) for c in range(NCORES)]
    return np.concatenate(outs, axis=1)             # [6, B, N]
